# revision 1
# baseline (speedup 1.0000x reference)
"""nn_BiRNNModel kernel for 8 Trainium2 NeuronCores.

2-layer bidirectional LSTM (B=32, T=512, H=1024), fp32.
Sharding: 8 cores = 2 directions x 4 batch-quarters; bwd cores run on
time-reversed input. Both layers run serially per core; no collectives.
"""
"""BiLSTM (2-layer, bidirectional) Trainium2 kernel.

Sharding: 8 cores = 2 directions x 4 batch-quarters (B_local=8). Each core runs
both layers of one direction on its batch slice; bwd cores get time-reversed
input. No collectives.

Per-core phases (each its own TileContext => implicit barrier between phases):
  A: Z0 = x @ Wih0^T + b0          (big matmul, xT tiles stationary)
  B: recurrent layer 0             (h^T stationary, Whh^T streamed via 4 PE
                                    column groups, one gate per group)
  C: Z1 = H0 @ Wih1^T + b1
  D: recurrent layer 1 -> OUT

Gate column order in all [*, 4H] tensors: [i | g | f | o] (psum column groups
at partition bases 0/32/64/96 respectively).
"""
import json
import numpy as np
from concourse import bass_utils as _bu

# walrus birsim scales with loop trip count (it simulates the whole
# execution); disable it for compile speed.
if not getattr(_bu, "_birsim_patched", False):
    _orig_run_command = _bu.run_command

    def _fast_run_command(cmd, **kw):
        cmd = [
            "--enable-birsim=false" if c == "--enable-birsim=true" else c
            for c in cmd
        ]
        return _orig_run_command(cmd, **kw)

    _bu.run_command = _fast_run_command
    _bu._birsim_patched = True
import concourse.bass as bass
import concourse.tile as tile
from concourse import mybir
from concourse.bass import ds
from concourse.bass_utils import run_bass_kernel_spmd
from concourse.masks import make_identity

F32 = mybir.dt.float32
AF = mybir.ActivationFunctionType

H = 1024
G4 = 4 * H
KT = H // 128  # 8 k-tiles
BL = 8         # local batch per core


def _proj_phase(nc, xsrc, w_d, b_d, z_d, T):
    """Z[b,t,:] = x[b,t,:] @ W^T + b.  xsrc: [KT,128,T,BL] (kc,kp,t,b)."""
    MT = T * BL // 128  # m-tiles (16 t x 8 b each)
    with tile.TileContext(nc) as tc:
        with (
            tc.tile_pool(name="pw", bufs=1) as pw,
            tc.tile_pool(name="px", bufs=3) as px,
            tc.tile_pool(name="pc", bufs=1) as pc,
            tc.tile_pool(name="pz", bufs=2, space="PSUM") as pz,
        ):
            w_sb = pw.tile([128, KT, G4], F32)
            nc.sync.dma_start(w_sb[:], w_d.rearrange("(kc k) j -> k kc j", k=128))
            b_sb = pc.tile([1, G4], F32)
            nc.sync.dma_start(b_sb[:], b_d[:])
            ones = pc.tile([1, 128], F32)
            nc.vector.memset(ones[:], 1.0)

            for m in range(MT):
                t0 = m * (128 // BL)
                x_t = px.tile([128, KT, 128], F32)
                nc.sync.dma_start(
                    x_t[:],
                    xsrc[:, :, t0 : t0 + 128 // BL, :].rearrange(
                        "kc kp t b -> kp kc (t b)"
                    ),
                )
                for nh in range(2):
                    p = pz.tile([128, 2048], F32)
                    for ns in range(4):
                        nsl = slice(ns * 512, ns * 512 + 512)
                        jsl = slice(nh * 2048 + ns * 512, nh * 2048 + ns * 512 + 512)
                        nc.tensor.matmul(
                            p[:, nsl], ones[:, :], b_sb[:, jsl],
                            start=True, stop=False,
                        )
                        for kc in range(KT):
                            nc.tensor.matmul(
                                p[:, nsl], x_t[:, kc, :], w_sb[:, kc, jsl],
                                start=False, stop=(kc == KT - 1),
                            )
                    zs = px.tile([128, 2048], F32, tag="zs")
                    nc.vector.tensor_copy(zs[:], p[:])
                    nc.sync.dma_start(
                        z_d[t0 : t0 + 128 // BL, :, nh * 2048 : nh * 2048 + 2048]
                        .rearrange("t b j -> (t b) j"),
                        zs[:],
                    )


def _recur_phase(nc, z_d, w_d, T, CH, hist_dst, hist_eng, use_for_i=True):
    """One LSTM layer over T steps. z_d: [BL,T,4H] (b,t,j). State/cell at lane
    block 0; psum gate blocks i@0 g@32 f@64 o@96."""
    n_chunks = T // CH
    with tile.TileContext(nc) as tc:
        with (
            tc.tile_pool(name="rw", bufs=1) as rw,
            tc.tile_pool(name="rst", bufs=1) as rst,
            tc.tile_pool(name="rz", bufs=2) as rz,
            tc.tile_pool(name="ra", bufs=1) as ra,
            tc.tile_pool(name="rc", bufs=1) as rc,
            tc.tile_pool(name="pg", bufs=2, space="PSUM") as pg,
            tc.tile_pool(name="pt", bufs=2, space="PSUM") as pt,
        ):
            w_sb = rw.tile([128, KT, G4], F32)
            nc.sync.dma_start(w_sb[:], w_d.rearrange("(kc k) j -> k kc j", k=128))

            id0 = rst.tile([8, 8], F32)          # identity @0 (z-add + transposes)
            make_identity(nc, id0[:])

            hist = rst.tile([128, KT, CH, BL], F32)  # h^T (kc, slot, b)
            nc.vector.memset(hist[:, :, CH - 1, :], 0.0)
            c_st = rst.tile([32, H], F32)        # c state rows 0:8
            nc.vector.memset(c_st[:8, :], 0.0)

            zflat = z_d.rearrange("t b j -> (t b) j")

            def body(ci):
                for j in range(CH):
                    pj = (j - 1) % CH
                    z_t = rz.tile([8, G4], F32, tag="zt")
                    zeng = nc.sync  # SWDGE (gpsimd) inside For_i breaks walrus sem-reset
                    zeng.dma_start(
                        z_t[:, :],
                        zflat[j * BL :, :][ds(ci * (CH * BL), BL), :],
                    )
                    gp = pg.tile([128, H], F32, tag="gates")
                    for g in range(4):
                        tp = (0, 32 * g)
                        ps = gp[32 * g : 32 * g + BL, :]
                        for ns in range(2):
                            nsl = slice(ns * 512, ns * 512 + 512)
                            nc.tensor.matmul(
                                ps[:, nsl], id0[:],
                                z_t[:, g * H + ns * 512 : g * H + ns * 512 + 512],
                                start=True, stop=False, tile_position=tp,
                            )
                        for kc in range(KT):
                            for ns in range(2):
                                nsl = slice(ns * 512, ns * 512 + 512)
                                nc.tensor.matmul(
                                    ps[:, nsl], hist[:, kc, pj, :],
                                    w_sb[:, kc, g * H + ns * 512 : g * H + ns * 512 + 512],
                                    start=False, stop=(kc == KT - 1),
                                    tile_position=tp,
                                )
                    # nonlinearities (lane-locked to psum block)
                    qa = ra.tile([128, H], F32, tag="acts")
                    nc.scalar.activation(qa[0:8, :], gp[0:8, :], AF.Sigmoid)      # i~
                    nc.scalar.activation(qa[32:40, :], gp[32:40, :], AF.Tanh)     # g~
                    nc.scalar.activation(qa[64:72, :], gp[64:72, :], AF.Sigmoid)  # f~
                    nc.scalar.activation(qa[96:104, :], gp[96:104, :], AF.Sigmoid)  # o~
                    # realign g~,f~,o~ to lane block 0 (SBUF->SBUF DMA)
                    wt = rc.tile([32, 4 * H], F32, tag="cellw")
                    mg, mf, mo = wt[:, 0:H], wt[:, H : 2 * H], wt[:, 2 * H : 3 * H]
                    fc = wt[:, 3 * H : 4 * H]
                    ig = wt[:, 0:H]       # overwrites mg after its read
                    tch = wt[:, H : 2 * H]  # overwrites mf after its read
                    nc.sync.dma_start(mg[0:8, :], qa[32:40, :])
                    nc.sync.dma_start(mf[0:8, :], qa[64:72, :])
                    nc.sync.dma_start(mo[0:8, :], qa[96:104, :])
                    nc.vector.tensor_mul(fc[0:8, :], mf[0:8, :], c_st[0:8, :])
                    nc.vector.tensor_mul(ig[0:8, :], qa[0:8, :], mg[0:8, :])
                    nc.vector.tensor_add(c_st[0:8, :], fc[0:8, :], ig[0:8, :])
                    nc.scalar.activation(tch[0:8, :], c_st[0:8, :], AF.Tanh)
                    h_t = ra.tile([32, H], F32, tag="ht")
                    nc.vector.tensor_mul(h_t[0:8, :], mo[0:8, :], tch[0:8, :])
                    # h^T via PE identity matmuls
                    tps = pt.tile([128, KT, BL], F32, tag="tp")
                    for kc in range(KT):
                        nc.tensor.matmul(
                            tps[:, kc, :], h_t[0:8, 128 * kc : 128 * kc + 128],
                            id0[:], start=True, stop=True,
                        )
                    nc.vector.tensor_copy(hist[:, :, j, :], tps[:])
                if True:
                    nc.sync.dma_start(
                        hist_dst.rearrange("kc kp t b -> kp kc t b")[
                            :, :, ds(ci * CH, CH), :
                        ],
                        hist[:],
                    )

            if use_for_i:
                with tc.For_i(0, n_chunks, 1) as ci:
                    body(ci)
            else:
                for ci in range(n_chunks):
                    body(ci)


def build_nc(T=512, CH=4, use_for_i=True):
    nc = bass.Bass()
    xT = nc.dram_tensor("xT", [KT, 128, T, BL], F32, kind="ExternalInput")
    wih0 = nc.dram_tensor("wih0", [H, G4], F32, kind="ExternalInput")
    whh0 = nc.dram_tensor("whh0", [H, G4], F32, kind="ExternalInput")
    wih1 = nc.dram_tensor("wih1", [H, G4], F32, kind="ExternalInput")
    whh1 = nc.dram_tensor("whh1", [H, G4], F32, kind="ExternalInput")
    b0 = nc.dram_tensor("b0", [1, G4], F32, kind="ExternalInput")
    b1 = nc.dram_tensor("b1", [1, G4], F32, kind="ExternalInput")
    z0 = nc.dram_tensor("z0", [T, BL, G4], F32, kind="Internal")
    z1 = nc.dram_tensor("z1", [T, BL, G4], F32, kind="Internal")
    h0t = nc.dram_tensor("h0t", [KT, 128, T, BL], F32, kind="Internal")
    out = nc.dram_tensor("out", [KT, 128, T, BL], F32, kind="ExternalOutput")

    _proj_phase(nc, xT, wih0, b0, z0, T)
    _recur_phase(nc, z0, whh0, T, CH, h0t, nc.sync, use_for_i=use_for_i)
    _proj_phase(nc, h0t, wih1, b1, z1, T)
    _recur_phase(nc, z1, whh1, T, CH, out, nc.gpsimd, use_for_i=use_for_i)

    # walrus in this container rejects >1 sync wait on CTRL instructions;
    # split them into single-wait drain chains.
    _install_multiwait_fix(nc)
    return nc


# --- BIR post-pass (inlined so kernel.py stays self-contained) ---
_SEQ_ENGINES = {"SP", "PE", "Activation", "DVE", "Pool", "SW", "ACT"}


def _split_multiwait(raw: bytes) -> bytes:
    m = json.loads(raw)
    changed = False
    for f in m.get("functions", []):
        for bb in f.get("blocks", []):
            insts = bb.get("instructions")
            if not insts:
                continue
            out = []
            for ins in insts:
                si = ins.get("sync_info")
                waits = (si or {}).get("on_wait") or []
                sem = [w for w in waits if w.get("sync_type") == "semaphore"]
                oth = [w for w in waits if w.get("sync_type") != "semaphore"]
                eng = ins.get("engine")
                if len(sem) + len(oth) > 1 and eng in _SEQ_ENGINES and len(sem) > 1:
                    keep = max(1 - len(oth), 0)
                    hoist = sem[: len(sem) - keep]
                    rest = sem[len(sem) - keep :]
                    changed = True
                    for j, w in enumerate(hoist):
                        out.append({
                            "debug": ins.get("debug"), "engine": eng,
                            "ins": [], "outs": [], "is_reset_sema": False,
                            "name": f"{ins['name']}_hw{j}", "opcode": "EventSemaphore",
                            "sync_info": {"on_update": [], "on_wait": [w]},
                        })
                    si["on_wait"] = oth + rest
                out.append(ins)
            bb["instructions"] = out
    return json.dumps(m).encode() if changed else raw


def _install_multiwait_fix(nc):
    orig = nc.to_json_bytes
    nc.to_json_bytes = lambda: _split_multiwait(orig())


# --- host side ---
_GP = None  # gate permutation cache


def _gate_perm():
    global _GP
    if _GP is None:
        idx = np.arange(G4).reshape(4, H)  # ref order i,f,g,o
        _GP = np.concatenate([idx[0], idx[2], idx[1], idx[3]])  # -> i,g,f,o
    return _GP


def make_in_maps(x, Wih_f, Whh_f, b_f, Wih_b, Whh_b, b_b, T=512):
    """Build the 8 per-core input maps."""
    gp = _gate_perm()
    B = x.shape[0]
    q = B // 4

    def wmap(Wih, Whh, b):
        return {
            "wih0": np.ascontiguousarray(Wih[0].T[:, gp]),
            "whh0": np.ascontiguousarray(Whh[0].T[:, gp]),
            "wih1": np.ascontiguousarray(Wih[1].T[:, gp]),
            "whh1": np.ascontiguousarray(Whh[1].T[:, gp]),
            "b0": np.ascontiguousarray(b[0][gp])[None, :],
            "b1": np.ascontiguousarray(b[1][gp])[None, :],
        }

    wf, wb = wmap(Wih_f, Whh_f, b_f), wmap(Wih_b, Whh_b, b_b)
    maps = []
    for c in range(8):
        d, bq = c // 4, c % 4
        xl = x[bq * q : (bq + 1) * q, :T, :]
        if d == 1:
            xl = xl[:, ::-1, :]
        xT = np.ascontiguousarray(xl.transpose(2, 1, 0)).reshape(KT, 128, T, q)
        m = {"xT": xT}
        m.update(wf if d == 0 else wb)
        maps.append(m)
    return maps


def assemble(results, T=512):
    q = BL
    fwd = np.empty((4 * q, T, H), np.float32)
    bwd = np.empty((4 * q, T, H), np.float32)
    for c in range(8):
        o = results[c]["out"].reshape(H, T, q).transpose(2, 1, 0)  # -> [BL,T,H]
        if c < 4:
            fwd[(c % 4) * q : (c % 4 + 1) * q] = o
        else:
            bwd[(c % 4) * q : (c % 4 + 1) * q] = o[:, ::-1, :]
    out = np.concatenate([fwd, bwd], axis=-1)
    return out, (fwd, bwd)


_NC_CACHE = {}


def _get_nc():
    if "nc" not in _NC_CACHE:
        _NC_CACHE["nc"] = build_nc(T=512)
    return _NC_CACHE["nc"]


def kernel(x, Wih_f, Whh_f, b_f, Wih_b, Whh_b, b_b):
    x = np.asarray(x, np.float32)
    args = [np.asarray(a, np.float32) for a in
            (Wih_f, Whh_f, b_f, Wih_b, Whh_b, b_b)]
    nc = _get_nc()
    maps = make_in_maps(x, *args, T=512)
    res = run_bass_kernel_spmd(nc, maps, core_ids=list(range(8)))
    return assemble(res.results, T=512)
